# revision 13
# baseline (speedup 1.0000x reference)
import sys

sys.path.insert(0, "/opt/trn_rl_repo")

import numpy as np

import concourse.bacc as bacc
import concourse.bass as bass
import concourse.mybir as mybir
import concourse.tile as tile
from concourse.bass_utils import run_bass_kernel_spmd

# Problem shapes (hardcoded per contract)
B = 4
NQ = 2048
NR = 16384
D = 64
K = 16

NCORES = 8
QPC = NQ // 2          # queries per core (each batch split across 2 cores)
NCHUNK = QPC // 128    # query chunks of 128 per core
HALF = 2048            # columns per psum tile (4 banks)
NHALF = NR // HALF     # 8 halves per chunk
MMN = 512              # matmul free dim (one PSUM bank)
TW = 512               # final slot-scan width
NROUND = 3             # top-8 rounds -> 24 slots
NSLOT = 8 * NROUND
SLOTSPAN = 32          # columns covered per slot (4 col-folds x 8 halves)

GAMMA = 327.0
DVEH = 3               # half converted by DVE instead of Act (engine balance)
CBIAS = 100.0

_prog_cache = {}


def _build_program(reps: int = 1):
    if reps in _prog_cache:
        return _prog_cache[reps]

    f32 = mybir.dt.float32
    f32r = mybir.dt.float32r
    u16 = mybir.dt.uint16
    u32 = mybir.dt.uint32
    amax = mybir.AluOpType.max

    nc = bacc.Bacc("TRN2", target_bir_lowering=False, debug=False, num_devices=NCORES)

    # psum = gamma*(C - d2) >= 0 for all plausible d2; negatives relu-clamped.
    # lhs rows 0..63 = 2*gamma*q^T, row 64 = -gamma, row 65 = gamma*(C - q2)
    # rhs rows 0..63 = ref^T,       row 64 = r2,     row 65 = 1.0
    lhs_d = nc.dram_tensor("lhs", [66, QPC], f32r, kind="ExternalInput")
    rhs_d = nc.dram_tensor("rhs", [66, NR], f32r, kind="ExternalInput")
    iota_d = nc.dram_tensor("iota", [128, TW], u32, kind="ExternalInput")
    outS_d = nc.dram_tensor("outS", [QPC, NSLOT], u32, kind="ExternalOutput")

    with tile.TileContext(nc) as tc:
        with (
            tc.tile_pool(name="consts", bufs=1) as cpool,
            tc.tile_pool(name="psum", bufs=2, space="PSUM") as ppool,
            tc.tile_pool(name="hbuf", bufs=2) as hpool,
            tc.tile_pool(name="merge", bufs=2) as mpool,
        ):
            lhs_t = cpool.tile([66, QPC], f32r)
            nc.sync.dma_start(lhs_t[:], lhs_d.ap())
            rhs_tiles = []
            for hh in range(NHALF):
                rt = cpool.tile([66, HALF], f32r, tag=f"rhs{hh}")
                nc.sync.dma_start(rt[:], rhs_d.ap()[:, hh * HALF:(hh + 1) * HALF])
                rhs_tiles.append(rt)
            iota_t = cpool.tile([128, TW], u32)
            nc.sync.dma_start(iota_t[:], iota_d.ap())

            for rep in range(reps):
              for c in range(NCHUNK):
                lhs_c = lhs_t[:, c * 128:(c + 1) * 128]
                pairs = [None] * 3
                g1 = g2 = h7 = r3 = None
                for h in range(NHALF):
                    ps = ppool.tile([128, HALF], f32, tag="ps")
                    for i in range(HALF // MMN):
                        nc.tensor.matmul(
                            ps[:, i * MMN:(i + 1) * MMN],
                            lhs_c,
                            rhs_tiles[h][:, i * MMN:(i + 1) * MMN],
                            start=True,
                            stop=True,
                        )
                    # convert to u16 coarse keys (monotone in -d2), clamp negatives
                    if h == DVEH:
                        # DVE window-2-reduces this half from PSUM directly:
                        # r3[j] = max(ps[2j], ps[2j+1]); joins below the tree.
                        r3 = hpool.tile([128, HALF // 2], u16, tag="r3")
                        nc.vector.tensor_reduce(
                            r3[:],
                            ps[:].rearrange("p (a b) -> p a b", b=2),
                            mybir.AxisListType.X,
                            amax,
                        )
                        continue
                    # halves 0,1 -> slices of AB; 2,4 -> CD; 5,6 -> EF; 7 alone
                    hi = h if h < DVEH else h - 1      # 0..6 among act halves
                    if hi < 6:
                        pair, sl = divmod(hi, 2)
                        if sl == 0:
                            pt = hpool.tile(
                                [128, 2 * HALF], u16, tag=f"pr{pair}"
                            )
                            pairs[pair] = pt
                        hbuf = pairs[pair][:, sl * HALF:(sl + 1) * HALF]
                    else:
                        h7 = hpool.tile([128, HALF], u16, tag="h7")
                        hbuf = h7[:]
                    nc.scalar.activation(
                        hbuf, ps[:], mybir.ActivationFunctionType.Relu
                    )
                    if hi == 3:
                        g1 = hpool.tile([128, 2 * HALF], u16, tag="g1")
                        nc.vector.tensor_tensor(
                            g1[:], pairs[0][:], pairs[1][:], amax
                        )
                    if hi == 5:
                        g2 = hpool.tile([128, HALF], u16, tag="g2")
                        nc.vector.tensor_tensor(
                            g2[:], pairs[2][:, 0:HALF], pairs[2][:, HALF:], amax
                        )
                # tree tail: g3 = fold(g1), g4 = max(g2, H7), U = max(g3, g4)
                g3 = hpool.tile([128, HALF], u16, tag="g3")
                nc.vector.tensor_tensor(g3[:], g1[:, 0:HALF], g1[:, HALF:], amax)
                g4 = hpool.tile([128, HALF], u16, tag="g4")
                nc.vector.tensor_tensor(g4[:], g2[:], h7[:], amax)
                ubuf = hpool.tile([128, HALF], u16, tag="u")
                nc.vector.tensor_tensor(ubuf[:], g3[:], g4[:], amax)

                # column folds 2048 -> 1024, merge r3, -> 512
                t1 = mpool.tile([128, 1024], u16, tag="t1")
                nc.vector.tensor_tensor(t1[:], ubuf[:, 0:1024], ubuf[:, 1024:2048], amax)
                t1b = mpool.tile([128, 1024], u16, tag="t1b")
                nc.vector.tensor_tensor(t1b[:], t1[:], r3[:], amax)
                tt = mpool.tile([128, TW], u16, tag="tt")
                nc.vector.tensor_tensor(tt[:], t1b[:, 0:TW], t1b[:, TW:1024], amax)

                # unique u32 keys: value*512 + slot_id  (exact: fits 2^24)
                t32 = mpool.tile([128, TW], u32, tag="t32")
                nc.vector.scalar_tensor_tensor(
                    t32[:], tt[:], 512.0, iota_t[:],
                    mybir.AluOpType.mult, mybir.AluOpType.add,
                )
                # 3 rounds of top-8 slot keys (no max_index needed)
                s24 = mpool.tile([128, NSLOT], u32, tag="s24")
                cur = t32
                for r in range(NROUND):
                    nc.vector.max(s24[:, r * 8:(r + 1) * 8], cur[:])
                    if r + 1 < NROUND:
                        nxt = mpool.tile([128, TW], u32, tag=f"mr_{r}")
                        nc.vector.match_replace(
                            nxt[:], s24[:, r * 8:(r + 1) * 8], cur[:], 0
                        )
                        cur = nxt

                r0, r1 = c * 128, (c + 1) * 128
                nc.sync.dma_start(outS_d.ap()[r0:r1, :], s24[:])

    nc.compile()
    _prog_cache[reps] = nc
    return nc


def kernel(ref: np.ndarray, query: np.ndarray):
    ref = np.asarray(ref, dtype=np.float32)
    query = np.asarray(query, dtype=np.float32)

    r2 = np.sum(ref * ref, axis=-1)                      # [B, NR]
    q2 = np.sum(query * query, axis=-1)                  # [B, NQ]
    refT = np.ascontiguousarray(ref.transpose(0, 2, 1))  # [B, D, NR]
    qT = np.ascontiguousarray(query.transpose(0, 2, 1))  # [B, D, NQ]

    nc = _build_program()

    iota_host = np.broadcast_to(
        np.arange(TW, dtype=np.uint32), (128, TW)
    ).copy()
    in_maps = []
    for core in range(NCORES):
        b, h = core // 2, core % 2
        lhs = np.empty((66, QPC), dtype=np.float32)
        lhs[0:D, :] = (2.0 * GAMMA) * qT[b][:, h * QPC:(h + 1) * QPC]
        lhs[D, :] = -GAMMA
        lhs[D + 1, :] = GAMMA * (CBIAS - q2[b, h * QPC:(h + 1) * QPC])
        rhs = np.empty((66, NR), dtype=np.float32)
        rhs[0:D, :] = refT[b]
        rhs[D, :] = r2[b]
        rhs[D + 1, :] = 1.0
        in_maps.append({"lhs": lhs, "rhs": rhs, "iota": iota_host})

    res = run_bass_kernel_spmd(nc, in_maps, core_ids=list(range(NCORES)))

    # host-side FAISS-style merge: decode candidate slots, exact-rescore, top-16
    # Act halves (h != DVEH): cols = j + 512m + 2048h; DVE half: window-2
    # reduced, cols = 2j + 2048*DVEH + {0, 1, 1024, 1025}.
    acth = np.array([h for h in range(NHALF) if h != DVEH])
    spanA = (512 * np.arange(4)[:, None] + HALF * acth[None, :]).ravel()
    spanB = HALF * DVEH + np.array([0, 1, 1024, 1025])
    Dout = np.empty((B, NQ, K), dtype=np.float32)
    Iout = np.empty((B, NQ, K), dtype=np.int64)
    for core in range(NCORES):
        b, h = core // 2, core % 2
        slots = (res.results[core]["outS"] & 511).astype(np.int64)  # [QPC, NSLOT]
        cols = np.concatenate(
            [slots[:, :, None] + spanA[None, None, :],
             2 * slots[:, :, None] + spanB[None, None, :]], axis=2
        ).reshape(QPC, -1)
        cols.sort(axis=1)
        dup = np.zeros(cols.shape, dtype=bool)
        dup[:, 1:] = cols[:, 1:] == cols[:, :-1]
        q_core = query[b][h * QPC:(h + 1) * QPC]                 # [QPC, D]
        q2_core = q2[b, h * QPC:(h + 1) * QPC]
        for s in range(0, QPC, 128):
            cs = cols[s:s + 128]                                 # [128, NCAND]
            g = ref[b][cs]                                       # [128, NCAND, D]
            dots = np.einsum('qd,qkd->qk', q_core[s:s + 128], g, optimize=True)
            d2 = q2_core[s:s + 128, None] + r2[b][cs] - 2.0 * dots
            d2 = np.maximum(d2, 0.0)
            d2[dup[s:s + 128]] = np.inf
            order = np.argsort(d2, axis=1, kind='stable')[:, :K]
            rows = np.arange(128)[:, None]
            Dout[b, h * QPC + s:h * QPC + s + 128] = np.sqrt(d2[rows, order])
            Iout[b, h * QPC + s:h * QPC + s + 128] = cs[rows, order]
    return (Dout, Iout)


# revision 14
# speedup vs baseline: 1.0002x; 1.0002x over previous
import sys

sys.path.insert(0, "/opt/trn_rl_repo")

import numpy as np

import concourse.bacc as bacc
import concourse.mybir as mybir
import concourse.tile as tile
from concourse.bass_utils import run_bass_kernel_spmd

# Problem shapes (hardcoded per contract)
B = 4
NQ = 2048
NR = 16384
D = 64
K = 16

NCORES = 8
QPC = NQ // 2          # queries per core (each batch split across 2 cores)
NCHUNK = QPC // 128    # query chunks of 128 per core
HALF = 2048            # columns per psum tile (4 banks)
NHALF = NR // HALF     # 8 halves per chunk
MMN = 512              # matmul free dim (one PSUM bank)
TW = 512               # final slot-scan width
NROUND = 3             # top-8 rounds -> 24 slots
NSLOT = 8 * NROUND     # 24 candidate slots per query; each spans 32 columns

GAMMA = 327.0
DVEH = 3               # half converted by DVE instead of Act (engine balance)
CBIAS = 100.0

_prog_cache = {}


def _build_program(reps: int = 1):
    if reps in _prog_cache:
        return _prog_cache[reps]

    f32 = mybir.dt.float32
    f32r = mybir.dt.float32r
    u16 = mybir.dt.uint16
    u32 = mybir.dt.uint32
    amax = mybir.AluOpType.max

    nc = bacc.Bacc("TRN2", target_bir_lowering=False, debug=False, num_devices=NCORES)

    # psum = gamma*(C - d2) >= 0 for all plausible d2; negatives relu-clamped.
    # lhs rows 0..63 = 2*gamma*q^T, row 64 = -gamma, row 65 = gamma*(C - q2)
    # rhs rows 0..63 = ref^T,       row 64 = r2,     row 65 = 1.0
    lhs_d = nc.dram_tensor("lhs", [66, QPC], f32r, kind="ExternalInput")
    rhs_d = nc.dram_tensor("rhs", [66, NR], f32r, kind="ExternalInput")
    iota_d = nc.dram_tensor("iota", [128, TW], u32, kind="ExternalInput")
    outS_d = nc.dram_tensor("outS", [QPC, NSLOT], u32, kind="ExternalOutput")

    with tile.TileContext(nc) as tc:
        with (
            tc.tile_pool(name="consts", bufs=1) as cpool,
            tc.tile_pool(name="psum", bufs=2, space="PSUM") as ppool,
            tc.tile_pool(name="hbuf", bufs=2) as hpool,
            tc.tile_pool(name="merge", bufs=2) as mpool,
        ):
            lhs_t = cpool.tile([66, QPC], f32r)
            nc.sync.dma_start(lhs_t[:], lhs_d.ap())
            rhs_tiles = []
            for hh in range(NHALF):
                rt = cpool.tile([66, HALF], f32r, tag=f"rhs{hh}")
                nc.sync.dma_start(rt[:], rhs_d.ap()[:, hh * HALF:(hh + 1) * HALF])
                rhs_tiles.append(rt)
            iota_t = cpool.tile([128, TW], u32)
            nc.sync.dma_start(iota_t[:], iota_d.ap())

            for rep in range(reps):
              for c in range(NCHUNK):
                lhs_c = lhs_t[:, c * 128:(c + 1) * 128]
                hlist = []
                r3 = None
                for h in range(NHALF):
                    ps = ppool.tile([128, HALF], f32, tag="ps")
                    for i in range(HALF // MMN):
                        nc.tensor.matmul(
                            ps[:, i * MMN:(i + 1) * MMN],
                            lhs_c,
                            rhs_tiles[h][:, i * MMN:(i + 1) * MMN],
                            start=True,
                            stop=True,
                        )
                    # convert to u16 coarse keys (monotone in -d2), clamp negatives
                    if h == DVEH:
                        # DVE window-2-reduces this half from PSUM directly:
                        # r3[j] = max(ps[2j], ps[2j+1]); joins below the tree.
                        r3 = hpool.tile([128, HALF // 2], u16, tag="r3")
                        nc.vector.tensor_reduce(
                            r3[:],
                            ps[:].rearrange("p (a b) -> p a b", b=2),
                            mybir.AxisListType.X,
                            amax,
                        )
                        continue
                    hbuf = hpool.tile([128, HALF], u16, tag=f"h{h % 2}")
                    nc.scalar.activation(
                        hbuf[:], ps[:], mybir.ActivationFunctionType.Relu
                    )
                    hlist.append(hbuf)
                # 7-half binary tree (half DVEH joins later at 1024 width)
                q = hlist
                while len(q) > 1:
                    nq = []
                    for i in range(0, len(q) - 1, 2):
                        ft = hpool.tile(
                            [128, HALF], u16, tag=f"f{len(q)}_{i // 2 % 2}"
                        )
                        nc.vector.tensor_tensor(ft[:], q[i][:], q[i + 1][:], amax)
                        nq.append(ft)
                    if len(q) % 2 == 1:
                        nq.append(q[-1])
                    q = nq
                ubuf = q[0]

                # column folds 2048 -> 1024, merge r3, -> 512
                t1 = mpool.tile([128, 1024], u16, tag="t1")
                nc.vector.tensor_tensor(t1[:], ubuf[:, 0:1024], ubuf[:, 1024:2048], amax)
                t1b = mpool.tile([128, 1024], u16, tag="t1b")
                nc.vector.tensor_tensor(t1b[:], t1[:], r3[:], amax)
                tt = mpool.tile([128, TW], u16, tag="tt")
                nc.vector.tensor_tensor(tt[:], t1b[:, 0:TW], t1b[:, TW:1024], amax)

                # unique u32 keys: value*512 + slot_id  (exact: fits 2^24)
                t32 = mpool.tile([128, TW], u32, tag="t32")
                nc.vector.scalar_tensor_tensor(
                    t32[:], tt[:], 512.0, iota_t[:],
                    mybir.AluOpType.mult, mybir.AluOpType.add,
                )
                # 3 rounds of top-8 slot keys (no max_index needed)
                s24 = mpool.tile([128, NSLOT], u32, tag="s24")
                cur = t32
                for r in range(NROUND):
                    nc.vector.max(s24[:, r * 8:(r + 1) * 8], cur[:])
                    if r + 1 < NROUND:
                        nxt = mpool.tile([128, TW], u32, tag=f"mr_{r}")
                        nc.vector.match_replace(
                            nxt[:], s24[:, r * 8:(r + 1) * 8], cur[:], 0
                        )
                        cur = nxt

                r0, r1 = c * 128, (c + 1) * 128
                nc.sync.dma_start(outS_d.ap()[r0:r1, :], s24[:])

    nc.compile()
    _prog_cache[reps] = nc
    return nc


def kernel(ref: np.ndarray, query: np.ndarray):
    ref = np.asarray(ref, dtype=np.float32)
    query = np.asarray(query, dtype=np.float32)

    r2 = np.sum(ref * ref, axis=-1)                      # [B, NR]
    q2 = np.sum(query * query, axis=-1)                  # [B, NQ]
    refT = np.ascontiguousarray(ref.transpose(0, 2, 1))  # [B, D, NR]
    qT = np.ascontiguousarray(query.transpose(0, 2, 1))  # [B, D, NQ]

    nc = _build_program()

    iota_host = np.broadcast_to(
        np.arange(TW, dtype=np.uint32), (128, TW)
    ).copy()
    in_maps = []
    for core in range(NCORES):
        b, h = core // 2, core % 2
        lhs = np.empty((66, QPC), dtype=np.float32)
        lhs[0:D, :] = (2.0 * GAMMA) * qT[b][:, h * QPC:(h + 1) * QPC]
        lhs[D, :] = -GAMMA
        lhs[D + 1, :] = GAMMA * (CBIAS - q2[b, h * QPC:(h + 1) * QPC])
        rhs = np.empty((66, NR), dtype=np.float32)
        rhs[0:D, :] = refT[b]
        rhs[D, :] = r2[b]
        rhs[D + 1, :] = 1.0
        in_maps.append({"lhs": lhs, "rhs": rhs, "iota": iota_host})

    res = run_bass_kernel_spmd(nc, in_maps, core_ids=list(range(NCORES)))

    # host-side FAISS-style merge: decode candidate slots, exact-rescore, top-16
    # Act halves (h != DVEH): cols = j + 512m + 2048h; DVE half: window-2
    # reduced, cols = 2j + 2048*DVEH + {0, 1, 1024, 1025}.
    acth = np.array([h for h in range(NHALF) if h != DVEH])
    spanA = (512 * np.arange(4)[:, None] + HALF * acth[None, :]).ravel()
    spanB = HALF * DVEH + np.array([0, 1, 1024, 1025])
    Dout = np.empty((B, NQ, K), dtype=np.float32)
    Iout = np.empty((B, NQ, K), dtype=np.int64)
    for core in range(NCORES):
        b, h = core // 2, core % 2
        slots = (res.results[core]["outS"] & 511).astype(np.int64)  # [QPC, NSLOT]
        cols = np.concatenate(
            [slots[:, :, None] + spanA[None, None, :],
             2 * slots[:, :, None] + spanB[None, None, :]], axis=2
        ).reshape(QPC, -1)
        cols.sort(axis=1)
        dup = np.zeros(cols.shape, dtype=bool)
        dup[:, 1:] = cols[:, 1:] == cols[:, :-1]
        q_core = query[b][h * QPC:(h + 1) * QPC]                 # [QPC, D]
        q2_core = q2[b, h * QPC:(h + 1) * QPC]
        for s in range(0, QPC, 128):
            cs = cols[s:s + 128]                                 # [128, NCAND]
            g = ref[b][cs]                                       # [128, NCAND, D]
            dots = np.einsum('qd,qkd->qk', q_core[s:s + 128], g, optimize=True)
            d2 = q2_core[s:s + 128, None] + r2[b][cs] - 2.0 * dots
            d2 = np.maximum(d2, 0.0)
            d2[dup[s:s + 128]] = np.inf
            order = np.argsort(d2, axis=1, kind='stable')[:, :K]
            rows = np.arange(128)[:, None]
            Dout[b, h * QPC + s:h * QPC + s + 128] = np.sqrt(d2[rows, order])
            Iout[b, h * QPC + s:h * QPC + s + 128] = cs[rows, order]
    return (Dout, Iout)


# revision 15
# speedup vs baseline: 1.0065x; 1.0063x over previous
import sys

sys.path.insert(0, "/opt/trn_rl_repo")

import numpy as np

import concourse.bacc as bacc
import concourse.mybir as mybir
import concourse.tile as tile
from concourse.bass_utils import run_bass_kernel_spmd

# Problem shapes (hardcoded per contract)
B = 4
NQ = 2048
NR = 16384
D = 64
K = 16

NCORES = 8
QPC = NQ // 2          # queries per core (each batch split across 2 cores)
NCHUNK = QPC // 128    # query chunks of 128 per core
HALF = 2048            # columns per psum tile (4 banks)
NHALF = NR // HALF     # 8 halves per chunk
MMN = 512              # matmul free dim (one PSUM bank)
TW = 512               # final slot-scan width
NROUND = 3             # top-8 rounds -> 24 slots
NSLOT = 8 * NROUND     # 24 candidate slots per query; each spans 32 columns

GAMMA = 327.0
DVEH = 3               # half converted by DVE instead of Act (engine balance)
CBIAS = 100.0

_prog_cache = {}


def _build_program(reps: int = 1):
    if reps in _prog_cache:
        return _prog_cache[reps]

    f32 = mybir.dt.float32
    f32r = mybir.dt.float32r
    u16 = mybir.dt.uint16
    u32 = mybir.dt.uint32
    amax = mybir.AluOpType.max

    nc = bacc.Bacc("TRN2", target_bir_lowering=False, debug=False, num_devices=NCORES)

    # psum = gamma*(C - d2) >= 0 for all plausible d2; negatives relu-clamped.
    # lhs rows 0..63 = 2*gamma*q^T, row 64 = -gamma, row 65 = gamma*(C - q2)
    # rhs rows 0..63 = ref^T,       row 64 = r2,     row 65 = 1.0
    lhs_d = nc.dram_tensor("lhs", [66, QPC], f32r, kind="ExternalInput")
    rhs_d = nc.dram_tensor("rhs", [66, NR], f32r, kind="ExternalInput")
    iota_d = nc.dram_tensor("iota", [128, TW], u32, kind="ExternalInput")
    outS_d = nc.dram_tensor("outS", [QPC, NSLOT], u32, kind="ExternalOutput")

    with tile.TileContext(nc) as tc:
        with (
            tc.tile_pool(name="consts", bufs=1) as cpool,
            tc.tile_pool(name="psum", bufs=2, space="PSUM") as ppool,
            tc.tile_pool(name="hbuf", bufs=3) as hpool,
            tc.tile_pool(name="merge", bufs=2) as mpool,
        ):
            lhs_t = cpool.tile([66, QPC], f32r)
            nc.sync.dma_start(lhs_t[:, 0:128], lhs_d.ap()[:, 0:128])
            nc.sync.dma_start(lhs_t[:, 128:QPC], lhs_d.ap()[:, 128:QPC])
            rhs_tiles = []
            for hh in range(NHALF):
                rt = cpool.tile([66, HALF], f32r, tag=f"rhs{hh}")
                if hh == 0:
                    # finer first-half loads so matmul 0 starts sooner
                    for qq in range(4):
                        nc.sync.dma_start(
                            rt[:, qq * MMN:(qq + 1) * MMN],
                            rhs_d.ap()[:, qq * MMN:(qq + 1) * MMN],
                        )
                else:
                    nc.sync.dma_start(rt[:], rhs_d.ap()[:, hh * HALF:(hh + 1) * HALF])
                rhs_tiles.append(rt)
            iota_t = cpool.tile([128, TW], u32)
            nc.sync.dma_start(iota_t[:], iota_d.ap())

            for rep in range(reps):
              for c in range(NCHUNK):
                lhs_c = lhs_t[:, c * 128:(c + 1) * 128]
                hlist = []
                r3 = None
                for h in range(NHALF):
                    ps = ppool.tile([128, HALF], f32, tag="ps")
                    for i in range(HALF // MMN):
                        nc.tensor.matmul(
                            ps[:, i * MMN:(i + 1) * MMN],
                            lhs_c,
                            rhs_tiles[h][:, i * MMN:(i + 1) * MMN],
                            start=True,
                            stop=True,
                        )
                    # convert to u16 coarse keys (monotone in -d2), clamp negatives
                    if h == DVEH:
                        # DVE window-2-reduces this half from PSUM directly:
                        # r3[j] = max(ps[2j], ps[2j+1]); joins below the tree.
                        r3 = hpool.tile([128, HALF // 2], u16, tag="r3")
                        nc.vector.tensor_reduce(
                            r3[:],
                            ps[:].rearrange("p (a b) -> p a b", b=2),
                            mybir.AxisListType.X,
                            amax,
                        )
                        continue
                    hbuf = hpool.tile([128, HALF], u16, tag=f"h{h % 2}")
                    nc.scalar.activation(
                        hbuf[:], ps[:], mybir.ActivationFunctionType.Relu
                    )
                    hlist.append(hbuf)
                # 7-half binary tree (half DVEH joins later at 1024 width)
                q = hlist
                while len(q) > 1:
                    nq = []
                    for i in range(0, len(q) - 1, 2):
                        ft = hpool.tile(
                            [128, HALF], u16, tag=f"f{len(q)}_{i // 2 % 2}"
                        )
                        nc.vector.tensor_tensor(ft[:], q[i][:], q[i + 1][:], amax)
                        nq.append(ft)
                    if len(q) % 2 == 1:
                        nq.append(q[-1])
                    q = nq
                ubuf = q[0]

                # column folds 2048 -> 1024, merge r3, -> 512
                t1 = mpool.tile([128, 1024], u16, tag="t1")
                nc.vector.tensor_tensor(t1[:], ubuf[:, 0:1024], ubuf[:, 1024:2048], amax)
                t1b = mpool.tile([128, 1024], u16, tag="t1b")
                nc.vector.tensor_tensor(t1b[:], t1[:], r3[:], amax)
                tt = mpool.tile([128, TW], u16, tag="tt")
                nc.vector.tensor_tensor(tt[:], t1b[:, 0:TW], t1b[:, TW:1024], amax)

                # unique u32 keys: value*512 + slot_id  (exact: fits 2^24)
                t32 = mpool.tile([128, TW], u32, tag="t32")
                nc.vector.scalar_tensor_tensor(
                    t32[:], tt[:], 512.0, iota_t[:],
                    mybir.AluOpType.mult, mybir.AluOpType.add,
                )
                # 3 rounds of top-8 slot keys (no max_index needed)
                s24 = mpool.tile([128, NSLOT], u32, tag="s24")
                cur = t32
                for r in range(NROUND):
                    nc.vector.max(s24[:, r * 8:(r + 1) * 8], cur[:])
                    if r + 1 < NROUND:
                        nxt = mpool.tile([128, TW], u32, tag=f"mr_{r}")
                        nc.vector.match_replace(
                            nxt[:], s24[:, r * 8:(r + 1) * 8], cur[:], 0
                        )
                        cur = nxt

                r0, r1 = c * 128, (c + 1) * 128
                nc.sync.dma_start(outS_d.ap()[r0:r1, :], s24[:])

    nc.compile()
    _prog_cache[reps] = nc
    return nc


def kernel(ref: np.ndarray, query: np.ndarray):
    ref = np.asarray(ref, dtype=np.float32)
    query = np.asarray(query, dtype=np.float32)

    r2 = np.sum(ref * ref, axis=-1)                      # [B, NR]
    q2 = np.sum(query * query, axis=-1)                  # [B, NQ]
    refT = np.ascontiguousarray(ref.transpose(0, 2, 1))  # [B, D, NR]
    qT = np.ascontiguousarray(query.transpose(0, 2, 1))  # [B, D, NQ]

    nc = _build_program()

    iota_host = np.broadcast_to(
        np.arange(TW, dtype=np.uint32), (128, TW)
    ).copy()
    in_maps = []
    for core in range(NCORES):
        b, h = core // 2, core % 2
        lhs = np.empty((66, QPC), dtype=np.float32)
        lhs[0:D, :] = (2.0 * GAMMA) * qT[b][:, h * QPC:(h + 1) * QPC]
        lhs[D, :] = -GAMMA
        lhs[D + 1, :] = GAMMA * (CBIAS - q2[b, h * QPC:(h + 1) * QPC])
        rhs = np.empty((66, NR), dtype=np.float32)
        rhs[0:D, :] = refT[b]
        rhs[D, :] = r2[b]
        rhs[D + 1, :] = 1.0
        in_maps.append({"lhs": lhs, "rhs": rhs, "iota": iota_host})

    res = run_bass_kernel_spmd(nc, in_maps, core_ids=list(range(NCORES)))

    # host-side FAISS-style merge: decode candidate slots, exact-rescore, top-16
    # Act halves (h != DVEH): cols = j + 512m + 2048h; DVE half: window-2
    # reduced, cols = 2j + 2048*DVEH + {0, 1, 1024, 1025}.
    acth = np.array([h for h in range(NHALF) if h != DVEH])
    spanA = (512 * np.arange(4)[:, None] + HALF * acth[None, :]).ravel()
    spanB = HALF * DVEH + np.array([0, 1, 1024, 1025])
    Dout = np.empty((B, NQ, K), dtype=np.float32)
    Iout = np.empty((B, NQ, K), dtype=np.int64)
    for core in range(NCORES):
        b, h = core // 2, core % 2
        slots = (res.results[core]["outS"] & 511).astype(np.int64)  # [QPC, NSLOT]
        cols = np.concatenate(
            [slots[:, :, None] + spanA[None, None, :],
             2 * slots[:, :, None] + spanB[None, None, :]], axis=2
        ).reshape(QPC, -1)
        cols.sort(axis=1)
        dup = np.zeros(cols.shape, dtype=bool)
        dup[:, 1:] = cols[:, 1:] == cols[:, :-1]
        q_core = query[b][h * QPC:(h + 1) * QPC]                 # [QPC, D]
        q2_core = q2[b, h * QPC:(h + 1) * QPC]
        for s in range(0, QPC, 128):
            cs = cols[s:s + 128]                                 # [128, NCAND]
            g = ref[b][cs]                                       # [128, NCAND, D]
            dots = np.einsum('qd,qkd->qk', q_core[s:s + 128], g, optimize=True)
            d2 = q2_core[s:s + 128, None] + r2[b][cs] - 2.0 * dots
            d2 = np.maximum(d2, 0.0)
            d2[dup[s:s + 128]] = np.inf
            order = np.argsort(d2, axis=1, kind='stable')[:, :K]
            rows = np.arange(128)[:, None]
            Dout[b, h * QPC + s:h * QPC + s + 128] = np.sqrt(d2[rows, order])
            Iout[b, h * QPC + s:h * QPC + s + 128] = cs[rows, order]
    return (Dout, Iout)


# revision 16
# speedup vs baseline: 1.0220x; 1.0155x over previous
import sys

sys.path.insert(0, "/opt/trn_rl_repo")

import numpy as np

import concourse.bacc as bacc
import concourse.mybir as mybir
import concourse.tile as tile
from concourse.bass_utils import run_bass_kernel_spmd

# Problem shapes (hardcoded per contract)
B = 4
NQ = 2048
NR = 16384
D = 64
K = 16

NCORES = 8
QPC = NQ // 2          # queries per core (each batch split across 2 cores)
NCHUNK = QPC // 128    # query chunks of 128 per core
HALF = 2048            # columns per psum tile (4 banks)
NHALF = NR // HALF     # 8 halves per chunk
MMN = 512              # matmul free dim (one PSUM bank)
TW = 512               # final slot-scan width
NROUND = 3             # top-8 rounds -> 24 slots
NSLOT = 8 * NROUND     # 24 candidate slots per query; each spans 32 columns

GAMMA = 327.0
DVEH = 3               # half converted by DVE instead of Act (engine balance)
CBIAS = 100.0

_prog_cache = {}


def _build_program(reps: int = 1):
    if reps in _prog_cache:
        return _prog_cache[reps]

    f32 = mybir.dt.float32
    f32r = mybir.dt.float32r
    u16 = mybir.dt.uint16
    u32 = mybir.dt.uint32
    amax = mybir.AluOpType.max

    nc = bacc.Bacc("TRN2", target_bir_lowering=False, debug=False, num_devices=NCORES)

    # psum = gamma*(C - d2) >= 0 for all plausible d2; negatives relu-clamped.
    # lhs rows 0..63 = 2*gamma*q^T, row 64 = -gamma, row 65 = gamma*(C - q2)
    # rhs rows 0..63 = ref^T,       row 64 = r2,     row 65 = 1.0
    lhs_d = nc.dram_tensor("lhs", [66, QPC], f32r, kind="ExternalInput")
    rhs_d = nc.dram_tensor("rhs", [66, NR], f32r, kind="ExternalInput")
    iota_d = nc.dram_tensor("iota", [128, TW], u32, kind="ExternalInput")
    outS_d = nc.dram_tensor("outS", [QPC, NSLOT], u32, kind="ExternalOutput")

    with tile.TileContext(nc) as tc:
        with (
            tc.tile_pool(name="consts", bufs=1) as cpool,
            tc.tile_pool(name="psum", bufs=2, space="PSUM") as ppool,
            tc.tile_pool(name="hbuf", bufs=3) as hpool,
            tc.tile_pool(name="merge", bufs=2) as mpool,
        ):
            lhs_t = cpool.tile([66, QPC], f32r)
            nc.sync.dma_start(lhs_t[:, 0:128], lhs_d.ap()[:, 0:128])
            nc.sync.dma_start(lhs_t[:, 128:QPC], lhs_d.ap()[:, 128:QPC])
            rhs_tiles = []
            for hh in range(NHALF):
                rt = cpool.tile([66, HALF], f32r, tag=f"rhs{hh}")
                if hh == 0:
                    # finer first-half loads so matmul 0 starts sooner
                    for qq in range(4):
                        nc.sync.dma_start(
                            rt[:, qq * MMN:(qq + 1) * MMN],
                            rhs_d.ap()[:, qq * MMN:(qq + 1) * MMN],
                        )
                else:
                    nc.sync.dma_start(rt[:], rhs_d.ap()[:, hh * HALF:(hh + 1) * HALF])
                rhs_tiles.append(rt)
            iota_t = cpool.tile([128, TW], u32)
            nc.sync.dma_start(iota_t[:], iota_d.ap())

            for rep in range(reps):
              for c in range(NCHUNK):
                lhs_c = lhs_t[:, c * 128:(c + 1) * 128]
                hlist = []
                r3 = None
                for h in range(NHALF):
                    ps = ppool.tile([128, HALF], f32, tag="ps")
                    for i in range(HALF // MMN):
                        nc.tensor.matmul(
                            ps[:, i * MMN:(i + 1) * MMN],
                            lhs_c,
                            rhs_tiles[h][:, i * MMN:(i + 1) * MMN],
                            start=True,
                            stop=True,
                        )
                    # convert to u16 coarse keys (monotone in -d2), clamp negatives
                    if h == DVEH:
                        # DVE window-2-reduces this half from PSUM directly:
                        # r3[j] = max(ps[2j], ps[2j+1]); joins below the tree.
                        r3 = hpool.tile([128, HALF // 2], u16, tag="r3")
                        nc.vector.tensor_reduce(
                            r3[:],
                            ps[:].rearrange("p (a b) -> p a b", b=2),
                            mybir.AxisListType.X,
                            amax,
                        )
                        continue
                    hbuf = hpool.tile([128, HALF], u16, tag=f"h{h % 2}")
                    nc.scalar.activation(
                        hbuf[:], ps[:], mybir.ActivationFunctionType.Relu
                    )
                    hlist.append(hbuf)
                # 7-half binary tree (half DVEH joins later at 1024 width)
                q = hlist
                while len(q) > 1:
                    nq = []
                    for i in range(0, len(q) - 1, 2):
                        ft = hpool.tile(
                            [128, HALF], u16, tag=f"f{len(q)}_{i // 2 % 2}"
                        )
                        nc.vector.tensor_tensor(ft[:], q[i][:], q[i + 1][:], amax)
                        nq.append(ft)
                    if len(q) % 2 == 1:
                        nq.append(q[-1])
                    q = nq
                ubuf = q[0]

                # column folds 2048 -> 1024, merge r3, -> 512
                t1 = mpool.tile([128, 1024], u16, tag="t1")
                nc.vector.tensor_tensor(t1[:], ubuf[:, 0:1024], ubuf[:, 1024:2048], amax)
                t1b = mpool.tile([128, 1024], u16, tag="t1b")
                nc.vector.tensor_tensor(t1b[:], t1[:], r3[:], amax)
                tt = mpool.tile([128, TW], u16, tag="tt")
                nc.vector.tensor_tensor(tt[:], t1b[:, 0:TW], t1b[:, TW:1024], amax)

                # unique u32 keys: value*512 + slot_id  (exact: fits 2^24)
                t32 = mpool.tile([128, TW], u32, tag="t32")
                nc.vector.scalar_tensor_tensor(
                    t32[:], tt[:], 512.0, iota_t[:],
                    mybir.AluOpType.mult, mybir.AluOpType.add,
                )
                # 24 slot keys: global top-8, then top-8 of each half of the
                # remainder (any true member has <=7 better slots left after
                # round 1, so it ranks <=8 within its half; keys are unique).
                s24 = mpool.tile([128, NSLOT], u32, tag="s24")
                nc.vector.max(s24[:, 0:8], t32[:])
                t2 = mpool.tile([128, TW], u32, tag="t2")
                nc.vector.match_replace(t2[:], s24[:, 0:8], t32[:], 0)
                nc.vector.max(s24[:, 8:16], t2[:, 0:TW // 2])
                nc.vector.max(s24[:, 16:24], t2[:, TW // 2:TW])

                r0, r1 = c * 128, (c + 1) * 128
                nc.sync.dma_start(outS_d.ap()[r0:r1, :], s24[:])

    nc.compile()
    _prog_cache[reps] = nc
    return nc


def kernel(ref: np.ndarray, query: np.ndarray):
    ref = np.asarray(ref, dtype=np.float32)
    query = np.asarray(query, dtype=np.float32)

    r2 = np.sum(ref * ref, axis=-1)                      # [B, NR]
    q2 = np.sum(query * query, axis=-1)                  # [B, NQ]
    refT = np.ascontiguousarray(ref.transpose(0, 2, 1))  # [B, D, NR]
    qT = np.ascontiguousarray(query.transpose(0, 2, 1))  # [B, D, NQ]

    nc = _build_program()

    iota_host = np.broadcast_to(
        np.arange(TW, dtype=np.uint32), (128, TW)
    ).copy()
    in_maps = []
    for core in range(NCORES):
        b, h = core // 2, core % 2
        lhs = np.empty((66, QPC), dtype=np.float32)
        lhs[0:D, :] = (2.0 * GAMMA) * qT[b][:, h * QPC:(h + 1) * QPC]
        lhs[D, :] = -GAMMA
        lhs[D + 1, :] = GAMMA * (CBIAS - q2[b, h * QPC:(h + 1) * QPC])
        rhs = np.empty((66, NR), dtype=np.float32)
        rhs[0:D, :] = refT[b]
        rhs[D, :] = r2[b]
        rhs[D + 1, :] = 1.0
        in_maps.append({"lhs": lhs, "rhs": rhs, "iota": iota_host})

    res = run_bass_kernel_spmd(nc, in_maps, core_ids=list(range(NCORES)))

    # host-side FAISS-style merge: decode candidate slots, exact-rescore, top-16
    # Act halves (h != DVEH): cols = j + 512m + 2048h; DVE half: window-2
    # reduced, cols = 2j + 2048*DVEH + {0, 1, 1024, 1025}.
    acth = np.array([h for h in range(NHALF) if h != DVEH])
    spanA = (512 * np.arange(4)[:, None] + HALF * acth[None, :]).ravel()
    spanB = HALF * DVEH + np.array([0, 1, 1024, 1025])
    Dout = np.empty((B, NQ, K), dtype=np.float32)
    Iout = np.empty((B, NQ, K), dtype=np.int64)
    for core in range(NCORES):
        b, h = core // 2, core % 2
        slots = (res.results[core]["outS"] & 511).astype(np.int64)  # [QPC, NSLOT]
        cols = np.concatenate(
            [slots[:, :, None] + spanA[None, None, :],
             2 * slots[:, :, None] + spanB[None, None, :]], axis=2
        ).reshape(QPC, -1)
        cols.sort(axis=1)
        dup = np.zeros(cols.shape, dtype=bool)
        dup[:, 1:] = cols[:, 1:] == cols[:, :-1]
        q_core = query[b][h * QPC:(h + 1) * QPC]                 # [QPC, D]
        q2_core = q2[b, h * QPC:(h + 1) * QPC]
        for s in range(0, QPC, 128):
            cs = cols[s:s + 128]                                 # [128, NCAND]
            g = ref[b][cs]                                       # [128, NCAND, D]
            dots = np.einsum('qd,qkd->qk', q_core[s:s + 128], g, optimize=True)
            d2 = q2_core[s:s + 128, None] + r2[b][cs] - 2.0 * dots
            d2 = np.maximum(d2, 0.0)
            d2[dup[s:s + 128]] = np.inf
            order = np.argsort(d2, axis=1, kind='stable')[:, :K]
            rows = np.arange(128)[:, None]
            Dout[b, h * QPC + s:h * QPC + s + 128] = np.sqrt(d2[rows, order])
            Iout[b, h * QPC + s:h * QPC + s + 128] = cs[rows, order]
    return (Dout, Iout)
